# revision 4
# baseline (speedup 1.0000x reference)
"""Lucas-Kanade optical flow kernel for 8 Trainium2 NeuronCores.

Sharding: columns across cores (256 out-cols each); within a core, partition p
holds rows [16p-1, 16p+18) so every stencil (2x2 derivative + 3x3 box) is a
free-dim shift. Host pre-gathers haloed tiles; edge rows/cols 0 and 2047 are
fixed up on the host (they need the reflect/zero-pad special cases).
"""

import sys

import numpy as np

sys.path.insert(0, "/opt/trn_rl_repo")

import orjson

import concourse.bass as bass
import concourse.mybir as mybir
from concourse.mybir import AluOpType as Op
from concourse.tile import TileContext
from concourse.bass_utils import run_bass_kernel_spmd

H = W = 2048
NCORE = 8
CPC = 256          # out cols per core
NCH = 4            # col chunks per core
CW = 64            # out cols per chunk
PIT = 68           # col slots per chunk (1 left halo + 64 + 2 right halo + pad)
NR = 19            # rows per partition (16 + 3 halo)
RPP = 16
FLAT = NR * PIT            # 1292 flat input elems per partition
NRES = FLAT - PIT - 1      # 1223 valid res elems
NC3 = NRES - 2             # 1221
NOUT = NRES - 2 - 2 * PIT  # 1085 valid solve elems
F32 = mybir.dt.float32

_WAIT_LIMIT = 1


def _split_waits(json_bytes):
    j = orjson.loads(json_bytes)
    n = 0
    for f in j["functions"]:
        for blk in f["blocks"]:
            out = []
            for ins in blk["instructions"]:
                si = ins.get("sync_info")
                ow = (si or {}).get("on_wait") or []
                while len(ow) > _WAIT_LIMIT:
                    take, ow = ow[:_WAIT_LIMIT], ow[_WAIT_LIMIT:]
                    n += 1
                    out.append({
                        "debug": ins.get("debug", 0),
                        "engine": ins["engine"],
                        "ins": [], "outs": [],
                        "name": f"I-wsplit{n}",
                        "opcode": "NoOp",
                        "sync_info": {"on_update": [], "on_wait": take},
                    })
                if si is not None:
                    si["on_wait"] = ow
                out.append(ins)
            blk["instructions"] = out
    return orjson.dumps(j)


def _build_nc():
    nc = bass.Bass()
    X = nc.declare_dram_parameter("X", [2, NCH, 128, NR, PIT], F32, isOutput=False)
    Y = nc.declare_dram_parameter("Y", [2, NCH, 128, RPP, CW], F32, isOutput=True)

    with TileContext(nc) as tc:
        with tc.tile_pool(name="io", bufs=4) as pio, \
             tc.tile_pool(name="wk", bufs=16) as pwk:
            for ch in range(NCH):
                i0 = pio.tile([128, FLAT], F32, tag="io", name=f"i0_{ch}")
                i1 = pio.tile([128, FLAT], F32, tag="io", name=f"i1_{ch}")
                nc.sync.dma_start(out=i0[:], in_=X[0, ch].rearrange("p r c -> p (r c)"))
                nc.sync.dma_start(out=i1[:], in_=X[1, ch].rearrange("p r c -> p (r c)"))

                _tc = [0]
                def T():
                    _tc[0] += 1
                    return pwk.tile([128, FLAT], F32, tag="wk", name=f"wk{ch}_{_tc[0]}")

                s = T(); nc.vector.tensor_add(s[:], i0[:], i1[:])
                d = T(); nc.vector.tensor_sub(d[:], i1[:], i0[:])
                n1 = FLAT - 1
                cd = T(); nc.vector.tensor_sub(cd[:, :n1], s[:, 1:], s[:, :n1])
                cs = T(); nc.vector.tensor_add(cs[:, :n1], s[:, 1:], s[:, :n1])
                dd = T(); nc.vector.tensor_add(dd[:, :n1], d[:, 1:], d[:, :n1])
                rx = T(); nc.vector.tensor_add(rx[:, :NRES], cd[:, :NRES], cd[:, PIT:PIT + NRES])
                ry = T(); nc.vector.tensor_sub(ry[:, :NRES], cs[:, PIT:PIT + NRES], cs[:, :NRES])
                rt = T(); nc.vector.tensor_add(rt[:, :NRES], dd[:, :NRES], dd[:, PIT:PIT + NRES])
                # products, folding in the reference's 0.5 derivative scaling
                xx = T(); nc.scalar.activation(xx[:, :NRES], rx[:, :NRES],
                                               mybir.ActivationFunctionType.Square, scale=0.5)
                yy = T(); nc.scalar.activation(yy[:, :NRES], ry[:, :NRES],
                                               mybir.ActivationFunctionType.Square, scale=0.5)
                xy = T(); nc.vector.scalar_tensor_tensor(xy[:, :NRES], rx[:, :NRES], 0.25,
                                                         ry[:, :NRES], Op.mult, Op.mult)
                xt = T(); nc.vector.scalar_tensor_tensor(xt[:, :NRES], rx[:, :NRES], 0.25,
                                                         rt[:, :NRES], Op.mult, Op.mult)
                yt = T(); nc.vector.scalar_tensor_tensor(yt[:, :NRES], ry[:, :NRES], 0.25,
                                                         rt[:, :NRES], Op.mult, Op.mult)
                # 3x3 box sums: cols then rows, all free-dim shifts
                boxed = []
                for idx, t in enumerate((xx, xy, yy, xt, yt)):
                    eng = nc.vector
                    ct = T(); eng.tensor_add(ct[:, :NC3], t[:, :NC3], t[:, 1:1 + NC3])
                    c3 = T(); eng.tensor_add(c3[:, :NC3], ct[:, :NC3], t[:, 2:2 + NC3])
                    bt = T(); eng.tensor_add(bt[:, :NOUT + PIT], c3[:, :NOUT + PIT],
                                             c3[:, PIT:NOUT + 2 * PIT])
                    r3 = T(); eng.tensor_add(r3[:, :NOUT], bt[:, :NOUT],
                                             c3[:, 2 * PIT:NOUT + 2 * PIT])
                    boxed.append(r3)
                bxx, bxy, byy, bxt, byt = boxed
                N = NOUT
                m = T(); nc.vector.tensor_mul(m[:, :N], bxx[:, :N], byy[:, :N])
                x2 = T(); nc.scalar.activation(x2[:, :N], bxy[:, :N],
                                               mybir.ActivationFunctionType.Square, scale=1.0)
                det = T(); nc.vector.tensor_sub(det[:, :N], m[:, :N], x2[:, :N])
                safe = T(); nc.vector.scalar_tensor_tensor(safe[:, :N], det[:, :N], 0.0,
                                                           det[:, :N], Op.is_equal, Op.add)
                rec = T()
                nc.vector.reciprocal(rec[:, :N], safe[:, :N])
                rm = T(); nc.vector.scalar_tensor_tensor(rm[:, :N], det[:, :N], 0.0,
                                                         rec[:, :N], Op.not_equal, Op.mult)
                a = T(); nc.vector.tensor_mul(a[:, :N], byy[:, :N], bxt[:, :N])
                b = T(); nc.vector.tensor_mul(b[:, :N], bxy[:, :N], byt[:, :N])
                nu = T(); nc.vector.tensor_sub(nu[:, :N], a[:, :N], b[:, :N])
                c2 = T(); nc.vector.tensor_mul(c2[:, :N], bxx[:, :N], byt[:, :N])
                e2 = T(); nc.vector.tensor_mul(e2[:, :N], bxy[:, :N], bxt[:, :N])
                nv = T(); nc.vector.tensor_sub(nv[:, :N], c2[:, :N], e2[:, :N])
                u = pwk.tile([128, RPP * PIT], F32, tag="wk", name=f"u{ch}")
                v = pwk.tile([128, RPP * PIT], F32, tag="wk", name=f"v{ch}")
                nc.vector.tensor_mul(u[:, :N], nu[:, :N], rm[:, :N])
                nc.vector.tensor_mul(v[:, :N], nv[:, :N], rm[:, :N])
                uv3 = u.rearrange("p (r c) -> p r c", c=PIT)
                vv3 = v.rearrange("p (r c) -> p r c", c=PIT)
                nc.sync.dma_start(out=Y[0, ch], in_=uv3[:, :, :CW])
                nc.sync.dma_start(out=Y[1, ch], in_=vv3[:, :, :CW])
    return nc


def _edge_fix(out, i0, i1):
    """Recompute out rows/cols {0, 2047} exactly on the host."""
    dt = np.float32
    def strip(r0, r1, c0, c1):
        # returns u,v for out rows [r0,r1) cols [c0,c1)
        pr0, pr1 = max(r0 - 2, 0), min(r1 + 2, H)
        pc0, pc1 = max(c0 - 2, 0), min(c1 + 2, W)
        a0 = i0[pr0:pr1 + 1 if pr1 < H else H, pc0:pc1]
        # full-frame reflect pad then slice: simpler and still cheap for strips
        p0 = np.pad(i0, ((0, 1), (0, 1)), mode='reflect')
        p1 = np.pad(i1, ((0, 1), (0, 1)), mode='reflect')
        def f2d(p, k):
            o = np.zeros((H, W), dt)
            for di in range(2):
                for dj in range(2):
                    o += k[di, dj] * p[di:di + H, dj:dj + W]
            return o
        return p0, p1, f2d
    p0 = np.pad(i0, ((0, 1), (0, 1)), mode='reflect')
    p1 = np.pad(i1, ((0, 1), (0, 1)), mode='reflect')
    fx = np.array([[-1, 1], [-1, 1]], dt); fy = np.array([[-1, -1], [1, 1]], dt)
    on = np.ones((2, 2), dt)
    def f2d(p, k):
        o = np.zeros((H, W), dt)
        for di in range(2):
            for dj in range(2):
                o += k[di, dj] * p[di:di + H, dj:dj + W]
        return o
    rx = dt(0.5) * (f2d(p0, fx) + f2d(p1, fx))
    ry = dt(0.5) * (f2d(p0, fy) + f2d(p1, fy))
    rt = dt(0.5) * (f2d(p1, on) - f2d(p0, on))
    def box3(x):
        p = np.pad(x, 1); o = np.zeros_like(x)
        for di in range(3):
            for dj in range(3):
                o += p[di:di + H, dj:dj + W]
        return o
    Ixx, Ixy, Iyy = box3(rx * rx), box3(rx * ry), box3(ry * ry)
    Ixt, Iyt = box3(rx * rt), box3(ry * rt)
    det = Ixx * Iyy - Ixy * Ixy
    ok = det != 0
    sd = np.where(ok, det, dt(1))
    u = np.where(ok, (Iyy * Ixt - Ixy * Iyt) / sd, 0).astype(dt)
    v = np.where(ok, (Ixx * Iyt - Ixy * Ixt) / sd, 0).astype(dt)
    for arr, ch in ((u, 0), (v, 1)):
        out[ch, 2047, :] = arr[2047, :]
        out[ch, :, 2047] = arr[:, 2047]
    out[:, 0, :] = 0
    out[:, :, 0] = 0
    return out


def kernel(input):
    input = np.ascontiguousarray(np.asarray(input, np.float32))
    i0, i1 = input[0, 0, 0], input[1, 0, 0]

    # padded frame: index g+1 holds row/col g; slot for g=2048 reflects g=2046
    pad = np.zeros((2, H + 3, W + 4), np.float32)
    pad[0, 1:H + 1, 1:W + 1] = i0
    pad[1, 1:H + 1, 1:W + 1] = i1
    pad[:, H + 1, :] = pad[:, H - 1, :]
    pad[:, :, W + 1] = pad[:, :, W - 1]

    in_maps = []
    for c in range(NCORE):
        Xc = np.empty((2, NCH, 128, NR, PIT), np.float32)
        for f in range(2):
            sw = np.lib.stride_tricks.sliding_window_view(pad[f], (NR, PIT))
            wins = sw[::RPP, ::CW]  # [128+, 32, NR, PIT] window starts
            Xc[f] = wins[:128, 4 * c:4 * c + 4].transpose(1, 0, 2, 3)
        in_maps.append({"X": Xc})

    nc = _build_nc()
    orig = type(nc).to_json_bytes
    type(nc).to_json_bytes = lambda self: _split_waits(orig(self))
    try:
        res = run_bass_kernel_spmd(nc, in_maps, core_ids=list(range(NCORE)))
    finally:
        type(nc).to_json_bytes = orig

    out = np.empty((2, H, W), np.float32)
    for c in range(NCORE):
        Yc = res.results[c]["Y"]  # [2, NCH, 128, RPP, CW]
        out[:, :, CPC * c:CPC * (c + 1)] = (
            Yc.transpose(0, 2, 3, 1, 4).reshape(2, H, CPC))
    return _edge_fix(out, i0, i1)
